# revision 39
# baseline (speedup 1.0000x reference)
"""KNN self-layer Trainium2 kernel (exact fp32 results).

Full computation: x [2, 1024, 64] f32 ->
  dist[b,i,j] = sum_f |x[b,i,f]-x[b,j,f]|  (L1)
  idx = top-17 smallest dist per (b,i)  (self included, ties by lowest j)
  out[b,i,f,k] = x[b, idx[b,i,k], f]   -> [2, 1024, 64, 17]

Sharding: 8 cores = 2 batches x 4 row-blocks of 256 rows. Each core gets the
full x[b] ("x_all") for the j/candidate side + gather, and its own 256-row
slice ("x_rows") for the i side. No cross-core comms. x_all is ROTATED on the
host so each core's own rows sit at j in [0, 256): the in-core symmetric
block position becomes core-independent (SPMD-safe), and since the output is
gathered x-rows the index rotation is invisible in the result.

In-core symmetry: tile 1 (rows 128..255) skips j in [0, 128) entirely — that
block equals the transpose of tile 0's columns [128, 256), mirrored into
tile 1's nd with one PE transpose before tile 0's match_replace runs
(-12.5% of tile-1 elementwise and matmul work). Tile 1 therefore runs a
7H:1D stream mix vs tile 0's 6H:2D (per-tile JOBS_TABLE).

Per-core algorithm (i handled in pairs: i = i0 + 2*u + q, q in {0,1}):
  - SBUF layout xtTdup[(q,f), j] = x_all[j, f]  (x^T duplicated in both
    partition halves, built via one batched strided load + PE transposes)
    and xiT[(q,f), u] = x_rows[2u+q, f].
  - |v| = 2*relu(v) - v: each pair u computes r[(q,f), j] = relu(x_all[j,f]
    - xi[f]) with ONE fused instr (DVE/Pool tensor_scalar(subtract, max) at
    fp32 2x mode, or ACT activation(Relu, bias=-xi)), spread across all three
    elementwise engines by a measured-cost schedule (JOBS).
  - TensorE reduces over f with shifted -2 weights (lhsT view of rwide):
    psum[2u+q, j] += -2*sum_f r. Two streams keep results EXACT while using
    the 1-cycle/row float32r matmul path for most pairs:
      "H" pairs: hi = f32r(r), lo = r - hi (telescoped split; hi+lo == r
        exactly), two f32r matmuls per jb;
      "D" pairs: one plain fp32 matmul (4 cyc/row) straight on r.
    The 6H:2D mix balances PE (~8.5us/8u) against the elementwise engines.
  - Rank-1 corrections psum[m,j] += sx[j] - si[m] (sx = sum_f x_all[j],
    si = sum_f x_rows[m]) close each accumulation group as one exact-fp32
    K=2 matmul per (tile, jb); operands built on-chip via small PE sums and
    SBUF->SBUF DMA assembly (engine APs need 32-aligned partition offsets).
  - ACT copies PSUM->SBUF; DVE max/max_index/match_replace x3 rounds gives
    the top-24 (descending) negdist indices; first 17 are the answer.
  - After each topk round the ready 8-index chunk is gathered (one indirect
    DMA per index: HW SWDGE ignores extra offsets, multi-offset APs gather
    consecutive rows and can crash) and ACT strided-copies the transpose
    [17,64]->[64,17]; one contiguous DMA per tile writes the result.
"""

import numpy as np

import concourse.bass as bass
from concourse import bacc
import concourse.mybir as mybir
from concourse import bass_utils
from concourse.bass import IndirectOffsetOnAxis
from concourse.masks import make_identity
from concourse.tile import TileContext

B = 2
N = 1024
F = 64
K1 = 17  # k+1 neighbors incl. self
NI = 256  # i-rows per core
P = 128
NCORES = 8
NEG_INF = -1.0e30

# Per-u stream mix: "H" = hi/lo split (3 elementwise passes, two 1-cyc/row
# f32r matmul streams), "D" = direct fp32 (1 pass, one 4-cyc/row fp32 matmul
# stream). The mix balances PE against the elementwise engines.
# Measured pass costs (ns): A relu 594/1038/1517 (v/s/g),
# B copy 594/1038/1517, C subtract 1127 (v) / 2127 (g, never s).
# Per-tile tables: tile 0 runs 6H:2D; tile 1 (whose j-range is 896 wide
# after the symmetry skip, so less PE work per pair) runs 7H:1D.
JOBS_TABLE = [
    [
        ("H", "s", "v", "v"), ("H", "s", "s", "g"),
        ("H", "v", "s", "v"), ("D", "g", None, None),
        ("H", "s", "v", "v"), ("H", "s", "s", "v"),
        ("H", "g", "g", "v"), ("D", "s", None, None),
    ],
    [
        ("H", "s", "v", "v"), ("H", "s", "s", "g"),
        ("H", "v", "s", "v"), ("H", "g", "v", "v"),
        ("H", "s", "v", "v"), ("H", "s", "s", "v"),
        ("H", "g", "g", "v"), ("D", "s", None, None),
    ],
]
# tail-op slot layout: (nd_copy, self_copy, round_base, round_stride,
# gather_stride, out_slot)
# slots >63 emit the whole chain after the next tile's compute stream.
# Sequential emission is the measured best LEGAL layout. CAUTION: interleaved
# slot layouts with round_stride < 5 reorder round r+1's max8 before round
# r's match_replace (the slot sort breaks the topk chain's intrinsic order) —
# a value race the timing-only sim cannot detect; it produced a fast but
# WRONG program. Legal (stride >= 5) interleaves all measured slower.
TAIL_SLOTS = (1000, 1001, 1002, 20, 1, 1200)
AD_BUFS = 8

_cached = {}
LAST_RESULT = None


def _make_schedule(d, a, p):
    """Largest-remainder interleave of 'v'(DVE)/'s'(ACT)/'g'(Pool) labels."""
    n = d + a + p
    counts = {"v": d, "s": a, "g": p}
    rates = {k: c / n for k, c in counts.items()}
    acc = {k: 0.0 for k in counts}
    left = dict(counts)
    out = []
    for _ in range(n):
        for k in acc:
            acc[k] += rates[k]
        k = max((kk for kk in "vsg" if left[kk] > 0), key=lambda kk: acc[kk])
        acc[k] -= 1.0
        left[k] -= 1
        out.append(k)
    return out


def _build():
    nc = bacc.Bacc("TRN2", target_bir_lowering=False, debug=False)

    x_all = nc.dram_tensor("x_all", [N, F], mybir.dt.float32, kind="ExternalInput")
    x_rows = nc.dram_tensor("x_rows", [NI, F], mybir.dt.float32, kind="ExternalInput")
    out_d = nc.dram_tensor(
        "out", [NI, F * K1], mybir.dt.float32, kind="ExternalOutput"
    )

    with TileContext(nc) as tc:
        with (
            tc.tile_pool(name="const", bufs=1) as constp,
            tc.tile_pool(name="xin", bufs=12) as xinp,
            tc.tile_pool(name="tpsum", bufs=3, space="PSUM") as tpsum,
            tc.tile_pool(name="tp0p", bufs=1, space="PSUM") as tp0p,
            tc.tile_pool(name="ad", bufs=AD_BUFS) as adp,
            tc.tile_pool(name="hi", bufs=AD_BUFS) as hip,
            tc.tile_pool(name="lo", bufs=AD_BUFS) as lop,
            tc.tile_pool(name="ndpsum", bufs=2, space="PSUM") as ndpsum,
            tc.tile_pool(name="ndsb", bufs=2) as ndsbp,
            tc.tile_pool(name="m8", bufs=4) as m8p,
            tc.tile_pool(name="idx", bufs=2) as idxp,
            tc.tile_pool(name="gat", bufs=2) as gatp,
            tc.tile_pool(name="og", bufs=2) as ogp,
        ):
            f32 = mybir.dt.float32
            f32r = mybir.dt.float32r

            ident = constp.tile([P, P], f32)
            make_identity(nc, ident[:])
            # Warm PE's view of the gpsimd semaphore (identity build) with a
            # dummy transpose, so each real transpose below carries only its
            # DMA wait — walrus allows a single sync-wait per LDWEIGHTS/DMA.
            ps0 = tp0p.tile([P, P], f32, tag="tp0")
            nc.tensor.transpose(ps0[:], ident[:], ident[:])

            # Batched input loads: one strided DMA each (8 resp. 2 dram
            # segments per partition) instead of 10 serial 625ns HWDGE gens.
            xr_big = xinp.tile([P, (NI // P) * F], f32, tag="xr")
            nc.sync.dma_start(
                xr_big[:].rearrange("p (t f) -> p t f", f=F),
                x_rows[:].rearrange("(t p) f -> p t f", p=P),
            )
            xa_big = xinp.tile([P, (N // P) * F], f32, tag="xa")
            for h in range(2):
                nc.sync.dma_start(
                    xa_big[:, h * 4 * F : (h + 1) * 4 * F].rearrange(
                        "p (t f) -> p t f", f=F
                    ),
                    x_all[h * 512 : (h + 1) * 512].rearrange(
                        "(t p) f -> p t f", p=P
                    ),
                )

            # xiT[(q,f), u] = x_rows[2u+q, f]  -> [128, 128]. Built first so
            # the absdiff producers can start as soon as xtTdup is ready.
            xiT = constp.tile([P, NI // 2], f32)
            ps2 = tpsum.tile([F, 512], f32, tag="tp")
            for t in range(NI // P):
                nc.tensor.transpose(
                    ps2[:, t * P : (t + 1) * P],
                    xr_big[:, t * F : (t + 1) * F],
                    ident[:],
                )
            for t in range(NI // P):
                # even local rows -> q=0 half, odd -> q=1 half
                pse = ps2[:, t * P : (t + 1) * P].rearrange(
                    "f (u two) -> f u two", two=2
                )
                dst = xiT[:, t * (P // 2) : (t + 1) * (P // 2)]
                nc.vector.tensor_copy(dst[0:F, :], pse[:, :, 0])
                nc.vector.tensor_copy(dst[F : 2 * F, :], pse[:, :, 1])

            # xtTdup[(q,f), j] = x_all[j, f] for q in {0,1}. 4 transposes
            # share one PSUM tile (disjoint ranges, PE program order) so no
            # PSUM slot is ever reused -> every instr carries <=1 sync wait.
            xtTdup = constp.tile([P, N], f32)
            for g in range(2):
                ps = tpsum.tile([F, 512], f32, tag="tp")
                for s in range(4):
                    t = 4 * g + s
                    nc.tensor.transpose(
                        ps[:, s * P : (s + 1) * P],
                        xa_big[:, t * F : (t + 1) * F],
                        ident[:],
                    )
                # q=0 half on ACT, q=1 half on DVE so the copies overlap
                nc.scalar.copy(xtTdup[0:F, g * 512 : (g + 1) * 512], ps[:])
                nc.vector.tensor_copy(
                    xtTdup[F : 2 * F, g * 512 : (g + 1) * 512], ps[:]
                )

            # Shifted-weight constant: rwide[(q,f), c] = -1 iff c == 126+q.
            # lhsT for pair u is the view rwide[:, 126-2u : 254-2u], so that
            # lhsT[k, m] = -1 iff m == 2u+q(k): matmul accumulates
            # psum[2u+q, j] += -sum_f ad[(q,f), j].
            # memset can't write f32r directly (ISA check); build in f32 and
            # round-convert with a copy. Weight -2: |v| = 2*relu(v) - v, so
            # negdist row m gets -2*sum_f relu plus rank-1 corrections below.
            rwide_f = constp.tile([P, 254], f32)
            nc.vector.memset(rwide_f[:], 0.0)
            nc.vector.memset(rwide_f[0:F, 126:127], -2.0)
            nc.vector.memset(rwide_f[F : 2 * F, 127:128], -2.0)
            rwide = constp.tile([P, 254], f32r)
            nc.vector.tensor_copy(rwide[:], rwide_f[:])

            # negated xi for the ACT relu path: relu(x + (-xi)).
            xiNeg = constp.tile([P, NI // 2], f32)
            nc.vector.tensor_scalar(
                xiNeg[:], xiT[:], -1.0, None, op0=mybir.AluOpType.mult
            )

            # Rank-1 correction operands (exact fp32):
            #   psum[m, j] += sx[j] - si[m],  sx[j]=sum_f x_all[j,f],
            #   si[m]=sum_f x_rows[m,f]  (m = 2u+q local row order).
            # One K=2 fp32 matmul per (tile, jb): lhsT=[ones; -si], rhs=[sx; ones].
            # Engine APs need 32-aligned partition offsets, so build each row
            # in a partition-0 tile and assemble the K=2 operands with
            # SBUF->SBUF DMAs (DMA has no partition alignment constraint).
            sxw = constp.tile([P, 1], f32)
            nc.vector.memset(sxw[:], 0.0)
            nc.vector.memset(sxw[0:F, :], 1.0)
            # siw columns 0 / 32 so the -si(q) rows land on partitions 0 / 32
            siw = constp.tile([P, 33], f32)
            nc.vector.memset(siw[:], 0.0)
            nc.vector.memset(siw[0:F, 0:1], -1.0)
            nc.vector.memset(siw[F : 2 * F, 32:33], -1.0)

            ones_row = constp.tile([1, N], f32)
            nc.vector.memset(ones_row[:], 1.0)
            nsi_row = constp.tile([1, NI], f32)
            sx_row = constp.tile([1, N], f32)
            # sx row via PE: psum[0, j] = sum_{k<F} xtTdup[k, j]
            for jb in range(N // 512):
                cps = tpsum.tile([F, 512], f32, tag="tp")
                nc.tensor.matmul(
                    cps[0:1, :],
                    lhsT=sxw[:, 0:1],
                    rhs=xtTdup[:, jb * 512 : (jb + 1) * 512],
                    start=True,
                    stop=True,
                )
                nc.scalar.copy(sx_row[0:1, jb * 512 : (jb + 1) * 512], cps[0:1, :])
            # -si rows via PE: psum[{0,32}, u] = -sum_f xiT[(q,f), u];
            # interleave (q, u) -> m = 2u+q with two strided copies.
            sips = tpsum.tile([F, 512], f32, tag="tp")
            nc.tensor.matmul(
                sips[0:33, 0 : NI // 2],
                lhsT=siw[:],
                rhs=xiT[:],
                start=True,
                stop=True,
            )
            nsiv = nsi_row[:].rearrange("one (u two) -> one u two", two=2)
            nc.vector.tensor_copy(nsiv[:, :, 0], sips[0:1, 0 : NI // 2])
            nc.vector.tensor_copy(nsiv[:, :, 1], sips[32:33, 0 : NI // 2])

            corr_lhsT = constp.tile([2, NI], f32)
            corr_rhs = constp.tile([2, N], f32)
            nc.vector.memset(corr_lhsT[0:1, :], 1.0)
            nc.scalar.copy(corr_rhs[0:1, :], sx_row[:])
            nc.sync.dma_start(corr_lhsT[1:2, :], nsi_row[:])
            nc.sync.dma_start(corr_rhs[1:2, :], ones_row[:])

            NT = NI // P  # i-tiles per core

            nd_sb = [None] * NT
            idx24 = [None] * NT

            def relu_pass(eng, out, uu, j0=0):
                """out = relu(x_all[:, j0:] - xi[uu]) on the given engine."""
                if eng == "v":
                    nc.vector.tensor_scalar(
                        out, xtTdup[:, j0:N], xiT[:, uu : uu + 1], 0.0,
                        op0=mybir.AluOpType.subtract, op1=mybir.AluOpType.max,
                    )
                elif eng == "g":
                    nc.gpsimd.tensor_scalar(
                        out, xtTdup[:, j0:N], xiT[:, uu : uu + 1], 0.0,
                        op0=mybir.AluOpType.subtract, op1=mybir.AluOpType.max,
                    )
                else:
                    nc.scalar.activation(
                        out, xtTdup[:, j0:N], mybir.ActivationFunctionType.Relu,
                        bias=xiNeg[:, uu : uu + 1],
                    )

            def copy_pass(eng, out, src):
                if eng == "v":
                    nc.vector.tensor_copy(out, src)
                elif eng == "g":
                    nc.gpsimd.tensor_copy(out, src)
                else:
                    nc.scalar.copy(out, src)

            def sub_pass(eng, out, a, b):
                if eng == "g":
                    nc.gpsimd.tensor_tensor(
                        out=out, in0=a, in1=b, op=mybir.AluOpType.subtract
                    )
                else:
                    nc.vector.tensor_tensor(
                        out=out, in0=a, in1=b, op=mybir.AluOpType.subtract
                    )

            # per-u job triple (A=relu fp32, B=hi f32r copy, C=lo subtract):
            # cycle of 8 triples balancing DVE ~1.68 : ACT ~0.96 : Pool ~0.66
            # engine rates; C never lands on ACT (no tensor_tensor there).
            JOBS = JOBS_TABLE

            def compute_tile(t, tail_ops=None):
                """Split-precision relu stream: exact r = hi(f32r) + lo(f32r)
                telescoped through two 1-cycle/row PE streams. tail_ops is a
                slot-ordered list of (u_slot, emit_fn) closures from the
                previous tile's topk/gather chain, interleaved here so each
                engine reaches them only once their dependencies are met."""
                ndps = ndpsum.tile([P, N], f32, tag="nd")
                # tile 1 skips j in [0, P): that block is the transpose of
                # tile 0's columns [P, 2P) (in-core distance symmetry), and
                # is mirrored into its nd tile by the tile-0 tail ops.
                j0 = 0 if t == 0 else P
                regions = [(j0, 512), (512, 1024)]
                ti = 0
                for u in range(P // 2):
                    while tail_ops and ti < len(tail_ops) and tail_ops[ti][0] <= u:
                        tail_ops[ti][1]()
                        ti += 1
                    uu = t * (P // 2) + u
                    tab = JOBS[t] if isinstance(JOBS[0], list) else JOBS
                    kind, ea, eb, ec = tab[u % len(tab)]
                    r = adp.tile([P, N], f32, tag="ad")
                    relu_pass(ea, r[:, j0:N], uu, j0)
                    if kind == "D":
                        lhsTf = rwide_f[:, 126 - 2 * u : 254 - 2 * u]
                        for r0, r1 in regions:
                            nc.tensor.matmul(
                                ndps[:, r0:r1],
                                lhsT=lhsTf,
                                rhs=r[:, r0:r1],
                                start=(u == 0),
                                stop=False,
                            )
                        continue
                    hi = hip.tile([P, N], f32r, tag="hi")
                    lo = lop.tile([P, N], f32r, tag="lo")
                    copy_pass(eb, hi[:, j0:N], r[:, j0:N])
                    sub_pass(ec, lo[:, j0:N], r[:, j0:N], hi[:, j0:N])
                    lhsT = rwide[:, 126 - 2 * u : 254 - 2 * u]
                    for r0, r1 in regions:
                        nc.tensor.matmul(
                            ndps[:, r0:r1],
                            lhsT=lhsT,
                            rhs=hi[:, r0:r1],
                            start=(u == 0),
                            stop=False,
                        )
                        nc.tensor.matmul(
                            ndps[:, r0:r1],
                            lhsT=lhsT,
                            rhs=lo[:, r0:r1],
                            start=False,
                            stop=False,
                        )
                while tail_ops and ti < len(tail_ops):
                    tail_ops[ti][1]()
                    ti += 1
                # exact fp32 rank-1 corrections close each accumulation group
                for r0, r1 in regions:
                    nc.tensor.matmul(
                        ndps[:, r0:r1],
                        lhsT=corr_lhsT[:, t * P : (t + 1) * P],
                        rhs=corr_rhs[:, r0:r1],
                        start=False,
                        stop=True,
                    )
                return ndps

            nd_tiles = [
                ndsbp.tile([P, N], f32, tag="nd_sb", name=f"nd_t{t}")
                for t in range(NI // P)
            ]

            def make_tail_ops(t, ndps):
                """Slot-ordered closures for tile t's psum copy, topk rounds,
                index-chunk gathers, output transposes and final DMA. k=0 is
                always self (dist 0 ranks first, ties by lowest j), so its
                row comes from the resident x_rows tile instead of a gather."""
                nd = nd_tiles[t]
                nd_sb[t] = nd
                idx = idxp.tile([P, 24], mybir.dt.uint32, tag="idx")
                idx24[t] = idx
                g = gatp.tile([P, K1 * F], f32, tag="g")
                o = ogp.tile([P, F * K1], f32, tag="o")
                gv = g[:].rearrange("p (kk f) -> p f kk", kk=K1)
                ov = o[:].rearrange("p (f kk) -> p f kk", kk=K1)
                xrv = xr_big[:, t * F : (t + 1) * F]
                s_nd, s_self, s_base, s_rstride, s_gstride, s_out = TAIL_SLOTS
                if t == 0:
                    ops = [(s_nd, lambda: nc.scalar.copy(nd[:], ndps[:]))]

                    # in-core symmetry mirror: nd1[c, q] = nd0[q, P+c]
                    # (dist(i0+q, i0+P+c) == dist(i0+P+c, i0+q)); must read nd
                    # before match_replace clobbers it (WAR tracked by tiles).
                    def do_mirror_t():
                        mps = tp0p.tile([P, P], f32, tag="tp0")
                        nc.tensor.transpose(mps[:], nd[:, P : 2 * P], ident[:])
                        nc.scalar.copy(nd_tiles[1][:, 0:P], mps[:])

                    ops.append((s_nd, do_mirror_t))
                else:
                    # j in [0, P) was mirrored in by tile 0's tail ops
                    ops = [
                        (s_nd, lambda: nc.scalar.copy(nd[:, P:N], ndps[:, P:N]))
                    ]
                # self row -> k=0 while the topk chain runs
                ops.append(
                    (s_self, lambda: nc.vector.tensor_copy(ov[:, :, 0:1].squeeze(-1), xrv))
                )

                def round_ops(r):
                    m8 = m8p.tile([P, 8], f32, tag="m8")
                    base = s_base + r * s_rstride

                    def do_max():
                        nc.vector.max(out=m8[:], in_=nd[:])

                    def do_idx():
                        nc.vector.max_index(
                            out=idx[:, r * 8 : (r + 1) * 8],
                            in_max=m8[:],
                            in_values=nd[:],
                        )

                    def do_mr():
                        nc.vector.match_replace(
                            out=nd[:], in_to_replace=m8[:], in_values=nd[:],
                            imm_value=NEG_INF,
                        )

                    ops.append((base, do_max))
                    ops.append((base + 2, do_idx))
                    if r < 2:
                        ops.append((base + 4, do_mr))
                    k0, k1 = max(1, r * 8), min(K1, r * 8 + 8)
                    for j, kk in enumerate(range(k0, k1)):
                        def do_gather(kk=kk):
                            nc.gpsimd.indirect_dma_start(
                                out=g[:, kk * F : (kk + 1) * F],
                                out_offset=None,
                                in_=x_all[:],
                                in_offset=IndirectOffsetOnAxis(
                                    ap=idx[:, kk : kk + 1], axis=0
                                ),
                            )
                        ops.append((base + 4 + s_gstride * j, do_gather))

                    def do_transpose(k0=k0, k1=k1):
                        nc.scalar.copy(ov[:, :, k0:k1], gv[:, :, k0:k1])

                    ops.append((base + 4 + s_gstride * (k1 - k0) + 2, do_transpose))

                for r in range(3):
                    round_ops(r)
                ops.append(
                    (s_out, lambda: nc.sync.dma_start(out_d[t * P : (t + 1) * P, :], o[:]))
                )
                ops.sort(key=lambda so: so[0])
                return ops

            # software pipeline: tile t's topk/gather chain is interleaved
            # into tile t+1's compute emission; the last tile's chain runs
            # standalone (the exposed drain).
            ndps_prev = compute_tile(0)
            for t in range(1, NT):
                ndps_prev = compute_tile(t, tail_ops=make_tail_ops(t - 1, ndps_prev))
            for _, fn in make_tail_ops(NT - 1, ndps_prev):
                fn()

    nc.finalize()
    return nc


def kernel(x):
    x = np.ascontiguousarray(np.asarray(x, dtype=np.float32))
    assert x.shape == (B, N, F)
    if "nc" not in _cached:
        _cached["nc"] = _build()
    nc = _cached["nc"]

    in_maps = []
    for c in range(NCORES):
        b, blk = c // 4, c % 4
        i0 = blk * NI
        in_maps.append(
            {
                # rotate so this core's own rows sit at j in [0, NI): makes
                # the symmetric-block mirror position core-independent (SPMD).
                # Output values are gathered rows, so the rotation is
                # invisible in the result.
                "x_all": np.ascontiguousarray(np.roll(x[b], -i0, axis=0)),
                "x_rows": np.ascontiguousarray(x[b, i0 : i0 + NI]),
            }
        )
    res = bass_utils.run_bass_kernel_spmd(nc, in_maps, core_ids=list(range(NCORES)))
    global LAST_RESULT
    LAST_RESULT = res
    full = np.empty((B, N, F, K1), np.float32)
    for c in range(NCORES):
        b, blk = c // 4, c % 4
        i0 = blk * NI
        full[b, i0 : i0 + NI] = res.results[c]["out"].reshape(NI, F, K1)
    return full
